# revision 14
# baseline (speedup 1.0000x reference)
"""Expert-parallel MoE kernel for 8 trn2 NeuronCores.

Strategy (expert-parallel, sparse):
  - Each core owns one expert (w1[e], w2[e] sharded via per-core input maps).
  - The SPMD program is identical on every core; per-core behavior comes from
    the data: the gate matrix columns are rotated per core so that column 0 is
    always "my expert".
  - On device, each core computes router logits for ALL tokens (fp32 — the
    min top2/top3 gap is ~5e-5, so reduced-precision routing would flip
    selections), derives its own combine weight per token, compacts the
    selected tokens into a dense buffer via an indirect-DMA row scatter
    (capacity C=1280 >= observed max count 1071), and runs the expert MLP on
    the compact tokens with float32r matmuls (full PE rate).
  - Host-side unshard: out[t] += y_c[slot_c[t]] for slots < C, summed over
    cores; router logits are taken from core 0 (whose rotation is identity).
"""

import numpy as np

import concourse.bass as bass
import concourse.bacc as bacc
import concourse.mybir as mybir
import concourse.tile as tile
from concourse.bass_utils import run_bass_kernel_spmd
from concourse.masks import make_identity, make_upper_triangular

# Problem shape (hardcoded; harness provides inputs of exactly this shape).
B, S, D = 2, 2048, 1024
E, H = 8, 2048
N = B * S            # 4096 tokens
P = 128
NT = N // P          # 32 token tiles
DC = D // P          # 8 contraction chunks over D
FT = (2 * H) // P    # 32 f-tiles over w1's output dim (2H)
HT = H // P          # 16 h-chunks over the contracted dim of w2
C = 1280             # per-expert token capacity (observed max count: 1071)
CT = C // P          # 10 compact tiles
ROWF = D + 8         # compact row: 1024 x-floats + w + pad (4128B, 32B aligned)
F32 = mybir.dt.float32
F32R = mybir.dt.float32r
I32 = mybir.dt.int32

NSPLIT = [(0, 512), (512, 1024), (1024, C)]  # moving-dim chunks for mm1 (<=512)


def r32(ap):
    return ap.bitcast(F32R)


def build_module():
    nc = bacc.Bacc(None, target_bir_lowering=False, debug=False)

    # Inputs (per-core maps may differ in content, not shape)
    xtt = nc.dram_tensor("xtt", [NT, P, D], F32, kind="ExternalInput")
    xrows = nc.dram_tensor("xrows", [N, D], F32, kind="ExternalInput")
    gwt = nc.dram_tensor("gwt", [P, DC * E], F32, kind="ExternalInput")
    w1t = nc.dram_tensor("w1t", [FT, P, DC * P], F32, kind="ExternalInput")
    w2 = nc.dram_tensor("w2", [H, D], F32, kind="ExternalInput")

    # Outputs
    logits_out = nc.dram_tensor("logits_out", [N, E], F32, kind="ExternalOutput")
    slot_out = nc.dram_tensor("slot_out", [P, NT], I32, kind="ExternalOutput")
    y_out = nc.dram_tensor("y_out", [C, D], F32, kind="ExternalOutput")

    # Internal compact buffer
    xc = nc.dram_tensor("xc", [C, ROWF], F32)

    from contextlib import ExitStack

    with tile.TileContext(nc) as tc:
        with (
            tc.tile_pool(name="consts", bufs=1) as consts,
            tc.tile_pool(name="route", bufs=1) as route,
            tc.tile_pool(name="stream", bufs=3) as stream,
            tc.tile_pool(name="xtiles", bufs=4) as xtiles,
            tc.tile_pool(name="big", bufs=1) as big,
            ExitStack() as phase_ad,
        ):
            rpsum = phase_ad.enter_context(
                tc.tile_pool(name="rpsum", bufs=2, space="PSUM")
            )
            tpsum = phase_ad.enter_context(
                tc.tile_pool(name="tpsum", bufs=2, space="PSUM")
            )
            # ---- constants ----
            ident = consts.tile([P, P], F32)
            make_identity(nc, ident[:])
            triu = consts.tile([P, P], F32)
            make_upper_triangular(nc, triu[:], val=1.0, diag=False)
            gw = consts.tile([P, DC, E], F32)
            nc.sync.dma_start(out=gw[:], in_=gwt[:].rearrange("p (c e) -> p c e", e=E))

            # ---- Phase A: router logits for all tokens (fp32) ----
            L = route.tile([P, NT, E], F32)          # logits, [p, i, e]
            for i in range(NT):
                xt = stream.tile([P, D], F32, tag="xtt")
                nc.sync.dma_start(out=xt[:], in_=xtt[i])
                xt3 = xt[:].rearrange("p (c t) -> p c t", c=DC)
                lps = rpsum.tile([P, E], F32, tag="lps")
                for dc in range(DC):
                    nc.tensor.matmul(
                        lps[:],
                        lhsT=xt3[:, dc, :],
                        rhs=gw[:, dc, :],
                        start=(dc == 0),
                        stop=(dc == DC - 1),
                    )
                nc.vector.tensor_copy(out=L[:, i, :], in_=lps[:])
            nc.sync.dma_start(
                out=logits_out[:].rearrange("(i p) e -> p i e", p=P), in_=L[:]
            )

            # ---- Phase B: top-2, combine weight, compaction slots ----
            M8 = route.tile([P, NT, E], F32)
            for i in range(NT):
                nc.vector.max(out=M8[:, i, :], in_=L[:, i, :])
            m1 = M8[:, :, 0]
            m2 = M8[:, :, 1]
            lc = L[:, :, 0]  # own expert's logit (rotation puts it at col 0)

            d1 = route.tile([P, NT], F32)
            d2 = route.tile([P, NT], F32)
            nc.vector.tensor_sub(d1[:], lc, m1)
            nc.vector.tensor_sub(d2[:], m2, m1)
            e1 = route.tile([P, NT], F32)
            e2 = route.tile([P, NT], F32)
            nc.scalar.activation(e1[:], d1[:], mybir.ActivationFunctionType.Exp)
            nc.scalar.activation(e2[:], d2[:], mybir.ActivationFunctionType.Exp)
            den = route.tile([P, NT], F32)
            nc.vector.tensor_scalar_add(den[:], e2[:], 1.0)
            rden = route.tile([P, NT], F32)
            nc.vector.reciprocal(rden[:], den[:])
            wq = route.tile([P, NT], F32)
            nc.vector.tensor_mul(wq[:], e1[:], rden[:])
            mask = route.tile([P, NT], F32)
            nc.vector.tensor_tensor(
                out=mask[:], in0=lc, in1=m2, op=mybir.AluOpType.is_ge
            )
            wv = route.tile([P, NT], F32)
            nc.vector.tensor_mul(wv[:], wq[:], mask[:])

            # slots: order (p, i); slot = (#selected in partitions < p)
            #                            + (#selected in row p, tiles < i)
            zeros = route.tile([P, NT], F32)
            nc.vector.memset(zeros[:], 0.0)
            incl = route.tile([P, NT], F32)
            nc.vector.tensor_tensor_scan(
                out=incl[:], data0=mask[:], data1=zeros[:], initial=0.0,
                op0=mybir.AluOpType.add, op1=mybir.AluOpType.add,
            )
            excl = route.tile([P, NT], F32)
            nc.vector.tensor_sub(excl[:], incl[:], mask[:])
            rowsum = route.tile([P, 1], F32)
            nc.vector.tensor_reduce(
                out=rowsum[:], in_=mask[:], axis=mybir.AxisListType.X,
                op=mybir.AluOpType.add,
            )
            pps = rpsum.tile([P, 1], F32, tag="pps")
            nc.tensor.matmul(pps[:], lhsT=triu[:], rhs=rowsum[:])
            pre = route.tile([P, 1], F32)
            nc.vector.tensor_copy(pre[:], pps[:])
            slot = route.tile([P, NT], F32)
            nc.vector.tensor_scalar_add(slot[:], excl[:], pre[:, 0:1])

            tgrid = route.tile([P, NT], I32)
            nc.gpsimd.iota(tgrid[:], pattern=[[P, NT]], base=0, channel_multiplier=1)
            tgrid_f = route.tile([P, NT], F32)
            nc.vector.tensor_copy(tgrid_f[:], tgrid[:])
            slot_f = route.tile([P, NT], F32)
            nc.vector.tensor_scalar_add(slot_f[:], tgrid_f[:], float(C))
            mask_i = route.tile([P, NT], I32)
            nc.vector.tensor_copy(mask_i[:], mask[:])
            nc.vector.copy_predicated(out=slot_f[:], mask=mask_i[:], data=slot[:])
            s32 = route.tile([P, NT], I32)
            nc.vector.tensor_copy(s32[:], slot_f[:])
            nc.sync.dma_start(out=slot_out[:], in_=s32[:])

            # ---- Phase C: zero xc, then scatter selected rows (x .. w) ----
            zrow = consts.tile([P, ROWF], F32)
            nc.vector.memset(zrow[:], 0.0)
            for j in range(CT):
                nc.sync.dma_start(out=xc[j * P:(j + 1) * P, :], in_=zrow[:])
            for i in range(NT):
                xt = xtiles.tile([P, ROWF], F32, tag="xin")
                nc.sync.dma_start(out=xt[:, :D], in_=xrows[i * P:(i + 1) * P, :])
                nc.vector.tensor_copy(out=xt[:, D:D + 1], in_=wv[:, i:i + 1])
                nc.vector.memset(xt[:, D + 1:], 0.0)
                nc.gpsimd.indirect_dma_start(
                    out=xc[:],
                    out_offset=bass.IndirectOffsetOnAxis(ap=s32[:, i:i + 1], axis=0),
                    in_=xt[:, :],
                    in_offset=None,
                    bounds_check=C - 1,
                    oob_is_err=False,
                )

            # ---- Phases D/E/F per capacity half (SBUF residency limit) ----
            # Half widths are 512/768 so every mm1 moving chunk is >=256
            # (full float32r rate).
            phase_ad.close()  # free the phase A-B PSUM banks
            wc = route.tile([P, CT], F32)
            WMAX = 768
            xcT = [
                big.tile([P, WMAX], F32R, tag=f"xcT{d}", name=f"xcT{d}")
                for d in range(DC)
            ]
            aT = [
                big.tile([P, WMAX], F32R, tag=f"aT{h}", name=f"aT{h}")
                for h in range(HT)
            ]
            for (h0, h1) in ((0, 512), (512, C)):
                W = h1 - h0
                nsplit = [(0, 512)] if W == 512 else [(0, 512), (512, W)]

                # -- D: load compact rows, transpose into xcT, grab weights --
                with tc.tile_pool(name=f"tp{h0}", bufs=2, space="PSUM") as tpsum:
                    for j in range(h0 // P, h1 // P):
                        o = j * P - h0
                        xt = xtiles.tile([P, ROWF], F32, tag="xcin")
                        nc.sync.dma_start(out=xt[:], in_=xc[j * P:(j + 1) * P, :])
                        nc.vector.tensor_copy(
                            out=wc[:, j:j + 1], in_=xt[:, D:D + 1]
                        )
                        for dc in range(DC):
                            tp = tpsum.tile([P, P], F32, tag="tp")
                            nc.tensor.transpose(
                                out=tp[:], in_=xt[:, dc * P:(dc + 1) * P],
                                identity=ident[:],
                            )
                            nc.vector.tensor_copy(
                                out=xcT[dc][:, o:o + P], in_=tp[:]
                            )

                # -- E: h^T = w1-chunks.T @ xcT (f32r), silu-gate -> aT --
                with tc.tile_pool(name=f"mm{h0}", bufs=1, space="PSUM") as mmpsum:
                    for jf in range(HT):  # 16 gate/linear f-tile pairs
                        psg = mmpsum.tile([P, W], F32, tag="psg")
                        psl = mmpsum.tile([P, W], F32, tag="psl")
                        for ps, f in ((psg, jf), (psl, jf + HT)):
                            w1f = stream.tile([P, DC * P], F32R, tag="w1f")
                            nc.sync.dma_start(out=w1f[:], in_=w1t[f].bitcast(F32R))
                            w1f3 = w1f[:].rearrange("p (c f) -> p c f", c=DC)
                            for dc in range(DC):
                                for (n0, n1) in nsplit:
                                    nc.tensor.matmul(
                                        ps[:, n0:n1],
                                        lhsT=w1f3[:, dc, :],
                                        rhs=xcT[dc][:, n0:n1],
                                        start=(dc == 0),
                                        stop=(dc == DC - 1),
                                    )
                        sg = stream.tile([P, W], F32, tag="sg")
                        nc.scalar.activation(
                            sg[:], psg[:], mybir.ActivationFunctionType.Sigmoid
                        )
                        nc.vector.tensor_mul(sg[:], sg[:], psg[:])
                        nc.vector.tensor_mul(aT[jf][:, :W], sg[:], psl[:])

                # -- F: y = a @ w2 (f32r), scale by own combine weight, store --
                with tc.tile_pool(name=f"yp{h0}", bufs=2, space="PSUM") as ypsum:
                    for t in range(h0 // P, h1 // P):
                        o = t * P - h0
                        psy = ypsum.tile([P, D], F32, tag="psy")
                        for h in range(HT):
                            w2c = stream.tile([P, D], F32R, tag="w2c")
                            nc.sync.dma_start(
                                out=w2c[:], in_=w2[h * P:(h + 1) * P, :].bitcast(F32R)
                            )
                            for (n0, n1) in ((0, 512), (512, 1024)):
                                nc.tensor.matmul(
                                    psy[:, n0:n1],
                                    lhsT=aT[h][:, o:o + P],
                                    rhs=w2c[:, n0:n1],
                                    start=(h == 0),
                                    stop=(h == HT - 1),
                                )
                        ysb = xtiles.tile([P, D], F32, tag="ysb")
                        nc.vector.tensor_scalar_mul(ysb[:], psy[:], wc[:, t:t + 1])
                        nc.sync.dma_start(
                            out=y_out[t * P:(t + 1) * P, :], in_=ysb[:]
                        )

    nc.compile()
    return nc


def make_in_maps(x, gate_w, w1, w2):
    """Build the 8 per-core input maps from the full tensors."""
    xf = np.ascontiguousarray(np.asarray(x, dtype=np.float32).reshape(N, D))
    gate_w = np.asarray(gate_w, dtype=np.float32)
    w1 = np.asarray(w1, dtype=np.float32)
    w2 = np.asarray(w2, dtype=np.float32)

    # x^T tiles: xtt[i, dp, dc*128 + t] = xf[i*128 + t, dc*128 + dp]
    xtt = np.ascontiguousarray(
        xf.reshape(NT, P, DC, P).transpose(0, 3, 2, 1).reshape(NT, P, D)
    )

    in_maps = []
    for c in range(E):
        rot = [(c + j) % E for j in range(E)]
        gwT = gate_w[rot].T  # [D, E], column 0 = expert c
        gwt = np.ascontiguousarray(
            gwT.reshape(DC, P, E).transpose(1, 0, 2).reshape(P, DC * E)
        )
        # w1t[f, dp, dc*128 + fc] = w1[c][dc*128 + dp, f*128 + fc]
        w1t = np.ascontiguousarray(
            w1[c].reshape(DC, P, FT, P).transpose(2, 1, 0, 3).reshape(FT, P, DC * P)
        )
        in_maps.append({
            "xtt": xtt,
            "xrows": xf,
            "gwt": gwt,
            "w1t": w1t,
            "w2": np.ascontiguousarray(w2[c]),
        })
    return in_maps


_cached = {}


def kernel(x, gate_w, w1, w2):
    key = "module"
    if key not in _cached:
        _cached[key] = build_module()
    nc = _cached[key]

    in_maps = make_in_maps(x, gate_w, w1, w2)
    res = run_bass_kernel_spmd(nc, in_maps, core_ids=list(range(E))).results

    out_flat = np.zeros((N, D), dtype=np.float32)
    for c in range(E):
        slots = res[c]["slot_out"]          # [P, NT], slot for token i*128+p
        slot_arr = np.ascontiguousarray(slots.T).reshape(N)
        sel = slot_arr < C
        y = res[c]["y_out"]                 # [C, D]
        out_flat[sel] += y[slot_arr[sel]]
    logits = res[0]["logits_out"]
    return out_flat.reshape(B, S, D), logits.reshape(B, S, E)


# revision 16
# speedup vs baseline: 1.2645x; 1.2645x over previous
"""Expert-parallel MoE kernel for 8 trn2 NeuronCores.

Strategy (expert-parallel, sparse):
  - Each core owns one expert (w1[e], w2[e] sharded via per-core input maps).
  - The SPMD program is identical on every core; per-core behavior comes from
    the data: the gate matrix columns are rotated per core so that column 0 is
    always "my expert".
  - On device, each core computes router logits for ALL tokens in full fp32
    (the min top2/top3 gap is ~5e-5, so reduced-precision routing would flip
    expert selections), derives its own combine weight per token, compacts the
    selected tokens into a dense buffer via a batched indirect-DMA row scatter
    (capacity C=1280 >= observed max count 1071), and runs the expert MLP on
    the compact tokens with bf16 matmuls (weights are pre-rounded to bf16 on
    the host; PSUM accumulation stays fp32).
  - Host-side unshard: out[t] += y_c[slot_c[t]] for slots < C, summed over
    cores; router logits are taken from core 0 (whose rotation is identity).
"""

import numpy as np
import ml_dtypes

import concourse.bass as bass
import concourse.bacc as bacc
import concourse.mybir as mybir
import concourse.tile as tile
from concourse.bass_utils import run_bass_kernel_spmd
from concourse.masks import make_identity, make_upper_triangular

# Problem shape (hardcoded; harness provides inputs of exactly this shape).
B, S, D = 2, 2048, 1024
E, H = 8, 2048
N = B * S            # 4096 tokens
P = 128
NT = N // P          # 32 token tiles
DC = D // P          # 8 contraction chunks over D
FT = (2 * H) // P    # 32 f-tiles over w1's output dim (2H)
HT = H // P          # 16 h-chunks over the contracted dim of w2
C = 1280             # per-expert token capacity (observed max count: 1071)
CT = C // P          # 10 compact tiles
ROWF = D + 8         # compact row: 1024 x floats + w + pad (4128B, 32B aligned)
NB = 8               # token tiles per scatter batch
TG = 512             # routing token-group width
F32 = mybir.dt.float32
BF16 = mybir.dt.bfloat16
I32 = mybir.dt.int32

HALVES = ((0, 512), (512, C))  # capacity halves for SBUF residency


def build_module():
    nc = bacc.Bacc(None, target_bir_lowering=False, debug=False)

    # Inputs (per-core maps may differ in content, not shape)
    xT = nc.dram_tensor("xT", [D, N], F32, kind="ExternalInput")
    xrows = nc.dram_tensor("xrows", [N, D], F32, kind="ExternalInput")
    gwt = nc.dram_tensor("gwt", [P, DC * E], F32, kind="ExternalInput")
    w1t = nc.dram_tensor("w1t", [FT, P, DC * P], BF16, kind="ExternalInput")
    w2 = nc.dram_tensor("w2", [H, D], BF16, kind="ExternalInput")

    # Outputs
    logits_out = nc.dram_tensor("logits_out", [N, E], F32, kind="ExternalOutput")
    slot_out = nc.dram_tensor("slot_out", [P, NT], I32, kind="ExternalOutput")
    y_out = nc.dram_tensor("y_out", [C, D], F32, kind="ExternalOutput")

    # Internal compact buffer
    xc = nc.dram_tensor("xc", [C, ROWF], F32)

    with tile.TileContext(nc) as tc:
        with (
            tc.tile_pool(name="consts", bufs=1) as consts,
            tc.tile_pool(name="route", bufs=1) as route,
            tc.tile_pool(name="stream", bufs=3) as stream,
            tc.tile_pool(name="xtiles", bufs=2) as xtiles,
            tc.tile_pool(name="big", bufs=1) as big,
        ):
            # ---- constants ----
            ident = consts.tile([P, P], F32)
            make_identity(nc, ident[:])
            identb = consts.tile([P, P], BF16)
            make_identity(nc, identb[:])
            triu = consts.tile([P, P], F32)
            make_upper_triangular(nc, triu[:], val=1.0, diag=False)
            gw = consts.tile([P, DC, E], F32)
            nc.sync.dma_start(out=gw[:], in_=gwt[:].rearrange("p (c e) -> p c e", e=E))

            rp_cm = tc.tile_pool(name="rpsum", bufs=2, space="PSUM")
            rpsum = rp_cm.__enter__()

            # ---- Phase A: router logits, fp32, gate stationary ----
            # logitsT[8, tok] = gw.T @ x ; accumulate over d-chunks, then
            # transpose [8,128] chunks into L[p, i, e].
            L = route.tile([P, NT, E], F32)          # logits, [p, i, e]
            for tg in range(N // TG):
                lt = rpsum.tile([E, TG], F32, tag="lt")
                for dc in range(DC):
                    xsl = stream.tile([P, TG], F32, tag="xsl", bufs=4)
                    nc.sync.dma_start(
                        out=xsl[:],
                        in_=xT[dc * P:(dc + 1) * P, tg * TG:(tg + 1) * TG],
                    )
                    nc.tensor.matmul(
                        lt[:], lhsT=gw[:, dc, :], rhs=xsl[:],
                        start=(dc == 0), stop=(dc == DC - 1),
                    )
                lts = route.tile([E, TG], F32, tag="lts", bufs=2)
                nc.vector.tensor_copy(lts[:], lt[:])
                for k in range(TG // P):
                    i = tg * (TG // P) + k
                    tp8 = rpsum.tile([P, E], F32, tag="tp8")
                    nc.tensor.transpose(
                        out=tp8[:], in_=lts[:, k * P:(k + 1) * P],
                        identity=ident[:E, :E],
                    )
                    nc.vector.tensor_copy(out=L[:, i, :], in_=tp8[:])
            nc.sync.dma_start(
                out=logits_out[:].rearrange("(i p) e -> p i e", p=P), in_=L[:]
            )

            # ---- Phase B: top-2, combine weight, compaction slots ----
            M8 = route.tile([P, NT, E], F32)
            for i in range(NT):
                nc.vector.max(out=M8[:, i, :], in_=L[:, i, :])
            m1 = M8[:, :, 0]
            m2 = M8[:, :, 1]
            lc = L[:, :, 0]  # own expert's logit (rotation puts it at col 0)

            d1 = route.tile([P, NT], F32)
            d2 = route.tile([P, NT], F32)
            nc.vector.tensor_sub(d1[:], lc, m1)
            nc.vector.tensor_sub(d2[:], m2, m1)
            e1 = route.tile([P, NT], F32)
            e2 = route.tile([P, NT], F32)
            nc.scalar.activation(e1[:], d1[:], mybir.ActivationFunctionType.Exp)
            nc.scalar.activation(e2[:], d2[:], mybir.ActivationFunctionType.Exp)
            den = route.tile([P, NT], F32)
            nc.vector.tensor_scalar_add(den[:], e2[:], 1.0)
            rden = route.tile([P, NT], F32)
            nc.vector.reciprocal(rden[:], den[:])
            wq = route.tile([P, NT], F32)
            nc.vector.tensor_mul(wq[:], e1[:], rden[:])
            mask = route.tile([P, NT], F32)
            nc.vector.tensor_tensor(
                out=mask[:], in0=lc, in1=m2, op=mybir.AluOpType.is_ge
            )
            wv = route.tile([P, NT], F32)
            nc.vector.tensor_mul(wv[:], wq[:], mask[:])

            # slots: order (p, i); slot = (#selected in partitions < p)
            #                            + (#selected in row p, tiles < i)
            zeros = route.tile([P, NT], F32)
            nc.vector.memset(zeros[:], 0.0)
            incl = route.tile([P, NT], F32)
            nc.vector.tensor_tensor_scan(
                out=incl[:], data0=mask[:], data1=zeros[:], initial=0.0,
                op0=mybir.AluOpType.add, op1=mybir.AluOpType.add,
            )
            excl = route.tile([P, NT], F32)
            nc.vector.tensor_sub(excl[:], incl[:], mask[:])
            rowsum = route.tile([P, 1], F32)
            nc.vector.tensor_reduce(
                out=rowsum[:], in_=mask[:], axis=mybir.AxisListType.X,
                op=mybir.AluOpType.add,
            )
            pps = rpsum.tile([P, 1], F32, tag="pps")
            nc.tensor.matmul(pps[:], lhsT=triu[:], rhs=rowsum[:])
            pre = route.tile([P, 1], F32)
            nc.vector.tensor_copy(pre[:], pps[:])
            slot = route.tile([P, NT], F32)
            nc.vector.tensor_scalar_add(slot[:], excl[:], pre[:, 0:1])

            tgrid = route.tile([P, NT], I32)
            nc.gpsimd.iota(tgrid[:], pattern=[[P, NT]], base=0, channel_multiplier=1)
            tgrid_f = route.tile([P, NT], F32)
            nc.vector.tensor_copy(tgrid_f[:], tgrid[:])
            slot_f = route.tile([P, NT], F32)
            nc.vector.tensor_scalar_add(slot_f[:], tgrid_f[:], float(C))
            mask_i = route.tile([P, NT], I32)
            nc.vector.tensor_copy(mask_i[:], mask[:])
            nc.vector.copy_predicated(out=slot_f[:], mask=mask_i[:], data=slot[:])
            s32 = route.tile([P, NT], I32)
            nc.vector.tensor_copy(s32[:], slot_f[:])
            nc.sync.dma_start(out=slot_out[:], in_=s32[:])

            # ---- Phase C: zero xc, then batched scatter of (x row .. w) ----
            zrow = consts.tile([P, ROWF], F32)
            nc.vector.memset(zrow[:], 0.0)
            for j in range(CT):
                nc.sync.dma_start(out=xc[j * P:(j + 1) * P, :], in_=zrow[:])
            for blk in range(NT // NB):
                mega = xtiles.tile([P, NB, ROWF], F32, tag="mega")
                nc.sync.dma_start(
                    out=mega[:, :, :D],
                    in_=xrows[blk * NB * P:(blk + 1) * NB * P, :].rearrange(
                        "(i p) d -> p i d", p=P
                    ),
                )
                nc.vector.tensor_copy(
                    out=mega[:, :, D], in_=wv[:, blk * NB:(blk + 1) * NB]
                )
                nc.vector.memset(mega[:, :, D + 1:], 0.0)
                # HW indirect DMA consumes one offset per partition, so one
                # scatter per 128-token tile.
                for ib in range(NB):
                    i = blk * NB + ib
                    nc.gpsimd.indirect_dma_start(
                        out=xc[:],
                        out_offset=bass.IndirectOffsetOnAxis(
                            ap=s32[:, i:i + 1], axis=0
                        ),
                        in_=mega[:, ib, :],
                        in_offset=None,
                        bounds_check=C - 1,
                        oob_is_err=False,
                    )

            rp_cm.__exit__(None, None, None)

            # ---- Phases D/E/F per capacity half (SBUF/PSUM residency) ----
            wc = route.tile([P, CT], F32)
            WMAX = max(h1 - h0 for h0, h1 in HALVES)
            xcT = [
                big.tile([P, WMAX], BF16, tag=f"xcT{d}", name=f"xcT{d}")
                for d in range(DC)
            ]
            aT = [
                big.tile([P, WMAX], BF16, tag=f"aT{h}", name=f"aT{h}")
                for h in range(HT)
            ]
            for (h0, h1) in HALVES:
                W = h1 - h0
                nsplit = [(0, 512)] if W == 512 else [(0, 512), (512, W)]

                # -- D: load compact rows, cast bf16, transpose into xcT --
                with tc.tile_pool(name=f"tp{h0}", bufs=2, space="PSUM") as tpsum:
                    for j in range(h0 // P, h1 // P):
                        o = j * P - h0
                        xt = xtiles.tile([P, ROWF], F32, tag="xcin")
                        nc.sync.dma_start(out=xt[:], in_=xc[j * P:(j + 1) * P, :])
                        nc.vector.tensor_copy(
                            out=wc[:, j:j + 1], in_=xt[:, D:D + 1]
                        )
                        xb = xtiles.tile([P, D], BF16, tag="xb")
                        nc.vector.tensor_copy(xb[:], xt[:, :D])
                        for dc in range(DC):
                            tp = tpsum.tile([P, P], BF16, tag="tp")
                            nc.tensor.transpose(
                                out=tp[:], in_=xb[:, dc * P:(dc + 1) * P],
                                identity=identb[:],
                            )
                            nc.vector.tensor_copy(
                                out=xcT[dc][:, o:o + P], in_=tp[:]
                            )

                # -- E: h^T = w1-chunks.T @ xcT (bf16), silu-gate -> aT --
                with tc.tile_pool(name=f"mm{h0}", bufs=1, space="PSUM") as mmpsum:
                    for jf in range(HT):  # 16 gate/linear f-tile pairs
                        psg = mmpsum.tile([P, W], F32, tag="psg")
                        psl = mmpsum.tile([P, W], F32, tag="psl")
                        for ps, f in ((psg, jf), (psl, jf + HT)):
                            w1f = stream.tile([P, DC * P], BF16, tag="w1f")
                            nc.sync.dma_start(out=w1f[:], in_=w1t[f])
                            w1f3 = w1f[:].rearrange("p (c f) -> p c f", c=DC)
                            for dc in range(DC):
                                for (n0, n1) in nsplit:
                                    nc.tensor.matmul(
                                        ps[:, n0:n1],
                                        lhsT=w1f3[:, dc, :],
                                        rhs=xcT[dc][:, n0:n1],
                                        start=(dc == 0),
                                        stop=(dc == DC - 1),
                                    )
                        sg = stream.tile([P, W], F32, tag="sg", bufs=2)
                        nc.scalar.activation(
                            sg[:], psg[:], mybir.ActivationFunctionType.Sigmoid
                        )
                        nc.vector.tensor_mul(sg[:], sg[:], psg[:])
                        nc.vector.tensor_mul(aT[jf][:, :W], sg[:], psl[:])

                # -- F: y = a @ w2 (bf16), scale by own combine weight, store --
                with tc.tile_pool(name=f"yp{h0}", bufs=2, space="PSUM") as ypsum:
                    for t in range(h0 // P, h1 // P):
                        o = t * P - h0
                        psy = ypsum.tile([P, D], F32, tag="psy")
                        for h in range(HT):
                            w2c = stream.tile([P, D], BF16, tag="w2c")
                            nc.sync.dma_start(
                                out=w2c[:], in_=w2[h * P:(h + 1) * P, :]
                            )
                            for (n0, n1) in ((0, 512), (512, 1024)):
                                nc.tensor.matmul(
                                    psy[:, n0:n1],
                                    lhsT=aT[h][:, o:o + P],
                                    rhs=w2c[:, n0:n1],
                                    start=(h == 0),
                                    stop=(h == HT - 1),
                                )
                        ysb = xtiles.tile([P, D], F32, tag="ysb")
                        nc.vector.tensor_scalar_mul(ysb[:], psy[:], wc[:, t:t + 1])
                        nc.sync.dma_start(
                            out=y_out[t * P:(t + 1) * P, :], in_=ysb[:]
                        )

    nc.compile()
    return nc


def make_in_maps(x, gate_w, w1, w2):
    """Build the 8 per-core input maps from the full tensors."""
    xf = np.ascontiguousarray(np.asarray(x, dtype=np.float32).reshape(N, D))
    gate_w = np.asarray(gate_w, dtype=np.float32)
    w1 = np.asarray(w1, dtype=np.float32)
    w2 = np.asarray(w2, dtype=np.float32)

    xT = np.ascontiguousarray(xf.T)

    in_maps = []
    for c in range(E):
        rot = [(c + j) % E for j in range(E)]
        gwT = gate_w[rot].T  # [D, E], column 0 = expert c
        gwt = np.ascontiguousarray(
            gwT.reshape(DC, P, E).transpose(1, 0, 2).reshape(P, DC * E)
        )
        # w1t[f, dp, dc*128 + fc] = w1[c][dc*128 + dp, f*128 + fc]
        w1t = np.ascontiguousarray(
            w1[c].reshape(DC, P, FT, P).transpose(2, 1, 0, 3).reshape(FT, P, DC * P)
        ).astype(ml_dtypes.bfloat16)
        in_maps.append({
            "xT": xT,
            "xrows": xf,
            "gwt": gwt,
            "w1t": w1t,
            "w2": np.ascontiguousarray(w2[c]).astype(ml_dtypes.bfloat16),
        })
    return in_maps


_cached = {}


def kernel(x, gate_w, w1, w2):
    key = "module"
    if key not in _cached:
        _cached[key] = build_module()
    nc = _cached[key]

    in_maps = make_in_maps(x, gate_w, w1, w2)
    res = run_bass_kernel_spmd(nc, in_maps, core_ids=list(range(E))).results

    out_flat = np.zeros((N, D), dtype=np.float32)
    for c in range(E):
        slots = res[c]["slot_out"]          # [P, NT], slot for token i*128+p
        slot_arr = np.ascontiguousarray(slots.T).reshape(N)
        sel = slot_arr < C
        y = res[c]["y_out"]                 # [C, D]
        out_flat[sel] += y[slot_arr[sel]]
    logits = res[0]["logits_out"]
    return out_flat.reshape(B, S, D), logits.reshape(B, S, E)


# revision 18
# speedup vs baseline: 1.6445x; 1.3005x over previous
"""Expert-parallel MoE kernel for 8 trn2 NeuronCores.

Strategy (expert-parallel, sparse):
  - Each core owns one expert (w1[e], w2[e] sharded via per-core input maps).
  - The SPMD program is identical on every core; per-core behavior comes from
    the data: the gate matrix columns are rotated per core so that column 0 is
    always "my expert".
  - On device, each core computes router logits for ALL tokens in full fp32
    (the min top2/top3 gap is ~5e-5, so reduced-precision routing would flip
    expert selections), derives its own combine weight per token, compacts the
    selected tokens into a dense buffer via indirect-DMA row scatters
    (capacity C=1280 >= observed max count 1071), and runs the expert MLP on
    the compact tokens with bf16 matmuls (weights pre-rounded to bf16 on the
    host; PSUM accumulation stays fp32).
  - Compaction slots are ordered (tile, partition) so each 128-token tile's
    scatter can fire as soon as that tile's routing is done — the serialized
    scatter chain overlaps the routing matmuls.
  - Host-side unshard: out[t] += y_c[slot_c[t]] for slots < C, summed over
    cores; router logits are taken from core 0 (whose rotation is identity).
"""

import numpy as np
import ml_dtypes

import concourse.bass as bass
import concourse.bacc as bacc
import concourse.mybir as mybir
import concourse.tile as tile
from concourse.bass_utils import run_bass_kernel_spmd
from concourse.masks import make_identity, make_upper_triangular

# Problem shape (hardcoded; harness provides inputs of exactly this shape).
B, S, D = 2, 2048, 1024
E, H = 8, 2048
N = B * S            # 4096 tokens
P = 128
NT = N // P          # 32 token tiles
DC = D // P          # 8 contraction chunks over D
FT = (2 * H) // P    # 32 f-tiles over w1's output dim (2H)
HT = H // P          # 16 h-chunks over the contracted dim of w2
C = 1280             # per-expert token capacity (observed max count: 1071)
CT = C // P          # 10 compact tiles
ROWF = D + 8         # compact row: 1024 x floats + w + pad (4128B, 32B aligned)
NB = 4               # token tiles per scatter x-load block
TG = 512             # routing token-group width
TPT = TG // P        # token tiles per routing group
F32 = mybir.dt.float32
BF16 = mybir.dt.bfloat16
I32 = mybir.dt.int32

GROUPS = ((0, 512), (512, 1024), (1024, C))  # capacity groups for the MLP


def build_module():
    nc = bacc.Bacc(None, target_bir_lowering=False, debug=False)

    # Inputs (per-core maps may differ in content, not shape)
    xT = nc.dram_tensor("xT", [D, N], F32, kind="ExternalInput")
    xrows = nc.dram_tensor("xrows", [N, D], F32, kind="ExternalInput")
    gwt = nc.dram_tensor("gwt", [P, DC * E], F32, kind="ExternalInput")
    w1t = nc.dram_tensor("w1t", [FT, P, DC * P], BF16, kind="ExternalInput")
    w2 = nc.dram_tensor("w2", [H, D], BF16, kind="ExternalInput")

    # Outputs
    logits_out = nc.dram_tensor("logits_out", [N, E], F32, kind="ExternalOutput")
    slot_out = nc.dram_tensor("slot_out", [P, NT], I32, kind="ExternalOutput")
    y_out = nc.dram_tensor("y_out", [C, D], F32, kind="ExternalOutput")

    # Internal compact buffer
    xc = nc.dram_tensor("xc", [C, ROWF], F32)

    with tile.TileContext(nc) as tc:
        with (
            tc.tile_pool(name="consts", bufs=1) as consts,
            tc.tile_pool(name="route", bufs=1) as route,
            tc.tile_pool(name="stream", bufs=3) as stream,
            tc.tile_pool(name="xtiles", bufs=2) as xtiles,
            tc.tile_pool(name="big", bufs=1) as big,
        ):
            # ---- constants ----
            ident = consts.tile([P, P], F32)
            make_identity(nc, ident[:])
            identb = consts.tile([P, P], BF16)
            make_identity(nc, identb[:])
            triub = consts.tile([P, P], BF16)
            make_upper_triangular(nc, triub[:], val=1.0, diag=False)
            onesb = consts.tile([P, P], BF16)
            nc.vector.memset(onesb[:], 1.0)
            gw = consts.tile([P, DC, E], F32)
            nc.sync.dma_start(out=gw[:], in_=gwt[:].rearrange("p (c e) -> p c e", e=E))
            w2all = [
                big.tile([P, D], BF16, tag=f"w2_{h}", name=f"w2_{h}")
                for h in range(HT)
            ]
            for h in range(HT):
                nc.sync.dma_start(out=w2all[h][:], in_=w2[h * P:(h + 1) * P, :])

            # zero-fill the compact buffer (one broadcast DMA)
            zrow = consts.tile([P, ROWF], F32)
            nc.vector.memset(zrow[:], 0.0)
            nc.sync.dma_start(
                out=xc[:].rearrange("(j p) r -> p j r", p=P),
                in_=zrow[:, None, :].to_broadcast([P, CT, ROWF]),
            )

            rp_cm = tc.tile_pool(name="rpsum", bufs=2, space="PSUM")
            rpsum = rp_cm.__enter__()

            # ---- routing + per-tile compaction, pipelined over token tiles --
            L = route.tile([P, NT, E], F32)          # logits, [p, i, e]
            wv = route.tile([P, NT], F32)            # own-expert combine weight
            s32 = route.tile([P, NT], I32)           # compaction slot per token
            tgrid = route.tile([P, NT], I32)
            nc.gpsimd.iota(tgrid[:], pattern=[[P, NT]], base=0, channel_multiplier=1)
            tgrid_f = route.tile([P, NT], F32)
            nc.vector.tensor_copy(tgrid_f[:], tgrid[:])
            runmask = route.tile([P, 1], BF16)       # selected-count so far/row
            nc.vector.memset(runmask[:], 0.0)

            megas = {}
            for blk in range(NT // NB):
                mega = xtiles.tile([P, NB, ROWF], F32, tag="mega", bufs=2,
                                   name=f"mega{blk}")
                nc.sync.dma_start(
                    out=mega[:, :, :D],
                    in_=xrows[blk * NB * P:(blk + 1) * NB * P, :].rearrange(
                        "(i p) d -> p i d", p=P
                    ),
                )
                nc.vector.memset(mega[:, :, D:], 0.0)
                megas[blk] = mega

            for tg in range(N // TG):
                # -- logits for 512 tokens: logitsT = gw.T @ x (fp32) --
                lt = rpsum.tile([E, TG], F32, tag="lt")
                for dc in range(DC):
                    xsl = stream.tile([P, TG], F32, tag="xsl", bufs=4)
                    nc.sync.dma_start(
                        out=xsl[:],
                        in_=xT[dc * P:(dc + 1) * P, tg * TG:(tg + 1) * TG],
                    )
                    nc.tensor.matmul(
                        lt[:], lhsT=gw[:, dc, :], rhs=xsl[:],
                        start=(dc == 0), stop=(dc == DC - 1),
                    )
                lts = route.tile([E, TG], F32, tag="lts", bufs=2)
                nc.vector.tensor_copy(lts[:], lt[:])
                M8 = route.tile([P, TPT, E], F32, tag="m8g", bufs=2)
                for k in range(TPT):
                    i = tg * TPT + k
                    tp8 = rpsum.tile([P, E], F32, tag="tp8")
                    nc.tensor.transpose(
                        out=tp8[:], in_=lts[:, k * P:(k + 1) * P],
                        identity=ident[:E, :E],
                    )
                    nc.vector.tensor_copy(out=L[:, i, :], in_=tp8[:])
                    nc.vector.max(out=M8[:, k, :], in_=L[:, i, :])

                # -- combine weights for this group (batched [P, TPT]) --
                g0 = tg * TPT
                lc = L[:, g0:g0 + TPT, 0]
                m1 = M8[:, :, 0]
                m2 = M8[:, :, 1]
                d1 = route.tile([P, TPT], F32, tag="d1", bufs=2)
                d2 = route.tile([P, TPT], F32, tag="d2", bufs=2)
                nc.vector.tensor_sub(d1[:], lc, m1)
                nc.vector.tensor_sub(d2[:], m2, m1)
                e1 = route.tile([P, TPT], F32, tag="e1", bufs=2)
                e2 = route.tile([P, TPT], F32, tag="e2", bufs=2)
                nc.scalar.activation(e1[:], d1[:], mybir.ActivationFunctionType.Exp)
                nc.scalar.activation(e2[:], d2[:], mybir.ActivationFunctionType.Exp)
                den = route.tile([P, TPT], F32, tag="den", bufs=2)
                nc.vector.tensor_scalar_add(den[:], e2[:], 1.0)
                rden = route.tile([P, TPT], F32, tag="rden", bufs=2)
                nc.vector.reciprocal(rden[:], den[:])
                wq = route.tile([P, TPT], F32, tag="wq", bufs=2)
                nc.vector.tensor_mul(wq[:], e1[:], rden[:])
                maskg = route.tile([P, TPT], F32, tag="maskg", bufs=2)
                nc.vector.tensor_tensor(
                    out=maskg[:], in0=lc, in1=m2, op=mybir.AluOpType.is_ge
                )
                nc.vector.tensor_mul(wv[:, g0:g0 + TPT], wq[:], maskg[:])

                # -- per-tile slots + scatter --
                for k in range(TPT):
                    i = g0 + k
                    mb = route.tile([P, 1], BF16, tag="mb", bufs=2)
                    nc.vector.tensor_copy(mb[:], maskg[:, k:k + 1])
                    sp = rpsum.tile([P, 1], F32, tag="sp")
                    # slot = (#sel in partitions<p of tile i) + (#sel tiles<i)
                    nc.tensor.matmul(sp[:], lhsT=triub[:], rhs=mb[:],
                                     start=True, stop=False)
                    nc.tensor.matmul(sp[:], lhsT=onesb[:], rhs=runmask[:],
                                     start=False, stop=True)
                    nc.vector.tensor_add(runmask[:], runmask[:], mb[:])
                    slot_f = route.tile([P, 1], F32, tag="slotf", bufs=2)
                    nc.vector.tensor_scalar_add(
                        slot_f[:], tgrid_f[:, i:i + 1], float(C)
                    )
                    mask_i = route.tile([P, 1], I32, tag="maski", bufs=2)
                    nc.vector.tensor_copy(mask_i[:], maskg[:, k:k + 1])
                    nc.vector.copy_predicated(
                        out=slot_f[:], mask=mask_i[:], data=sp[:]
                    )
                    nc.vector.tensor_copy(s32[:, i:i + 1], slot_f[:])
                    # write own combine weight into the staged x row, scatter
                    mega = megas[i // NB]
                    nc.vector.tensor_copy(
                        out=mega[:, i % NB, D:D + 1], in_=wv[:, i:i + 1]
                    )
                    nc.gpsimd.indirect_dma_start(
                        out=xc[:],
                        out_offset=bass.IndirectOffsetOnAxis(
                            ap=s32[:, i:i + 1], axis=0
                        ),
                        in_=mega[:, i % NB, :],
                        in_offset=None,
                        bounds_check=C - 1,
                        oob_is_err=False,
                    )

            nc.sync.dma_start(
                out=logits_out[:].rearrange("(i p) e -> p i e", p=P), in_=L[:]
            )
            nc.sync.dma_start(out=slot_out[:], in_=s32[:])
            rp_cm.__exit__(None, None, None)

            # ---- MLP phases per capacity group (PSUM residency) ----
            wc = route.tile([P, CT], F32)
            xcT = {}
            aT = {}
            for gi, (h0, h1) in enumerate(GROUPS):
                W = h1 - h0
                xcT[gi] = [
                    big.tile([P, W], BF16, tag=f"xcT{gi}_{d}", name=f"xcT{gi}_{d}")
                    for d in range(DC)
                ]
                aT[gi] = [
                    big.tile([P, W], BF16, tag=f"aT{gi}_{h}", name=f"aT{gi}_{h}")
                    for h in range(HT)
                ]

            tp_cm = tc.tile_pool(name="tpsum", bufs=2, space="PSUM")
            tpsum = tp_cm.__enter__()
            for gi, (h0, h1) in enumerate(GROUPS):
                W = h1 - h0

                # -- D: load compact rows, cast bf16, transpose into xcT --
                for j in range(h0 // P, h1 // P):
                    o = j * P - h0
                    xt = xtiles.tile([P, ROWF], F32, tag="xcin", bufs=2)
                    nc.sync.dma_start(out=xt[:], in_=xc[j * P:(j + 1) * P, :])
                    nc.vector.tensor_copy(out=wc[:, j:j + 1], in_=xt[:, D:D + 1])
                    xb = xtiles.tile([P, D], BF16, tag="xb", bufs=2)
                    nc.vector.tensor_copy(xb[:], xt[:, :D])
                    for dc in range(DC):
                        tp = tpsum.tile([P, P], BF16, tag="tp")
                        nc.tensor.transpose(
                            out=tp[:], in_=xb[:, dc * P:(dc + 1) * P],
                            identity=identb[:],
                        )
                        nc.vector.tensor_copy(out=xcT[gi][dc][:, o:o + P], in_=tp[:])

                # -- E: h^T = w1-chunks.T @ xcT (bf16), silu-gate -> aT --
                with tc.tile_pool(name=f"mm{gi}", bufs=2, space="PSUM") as mmpsum:
                    for jf in range(HT):  # 16 gate/linear f-tile pairs
                        psg = mmpsum.tile([P, W], F32, tag="psg")
                        psl = mmpsum.tile([P, W], F32, tag="psl")
                        for ps, f in ((psg, jf), (psl, jf + HT)):
                            w1f = stream.tile([P, DC * P], BF16, tag="w1f")
                            nc.sync.dma_start(out=w1f[:], in_=w1t[f])
                            w1f3 = w1f[:].rearrange("p (c f) -> p c f", c=DC)
                            for dc in range(DC):
                                nc.tensor.matmul(
                                    ps[:],
                                    lhsT=w1f3[:, dc, :],
                                    rhs=xcT[gi][dc][:],
                                    start=(dc == 0),
                                    stop=(dc == DC - 1),
                                )
                        sg = stream.tile([P, W], F32, tag="sg", bufs=2)
                        nc.scalar.activation(
                            sg[:], psg[:], mybir.ActivationFunctionType.Sigmoid
                        )
                        nc.vector.tensor_mul(sg[:], sg[:], psg[:])
                        nc.vector.tensor_mul(aT[gi][jf][:], sg[:], psl[:])

                # -- F: y = a @ w2 (bf16), scale by own combine weight, store --
                with tc.tile_pool(name=f"yp{gi}", bufs=1, space="PSUM") as ypsum:
                    for t in range(h0 // P, h1 // P):
                        o = t * P - h0
                        psy = ypsum.tile([P, D], F32, tag="psy")
                        for h in range(HT):
                            for (n0, n1) in ((0, 512), (512, 1024)):
                                nc.tensor.matmul(
                                    psy[:, n0:n1],
                                    lhsT=aT[gi][h][:, o:o + P],
                                    rhs=w2all[h][:, n0:n1],
                                    start=(h == 0),
                                    stop=(h == HT - 1),
                                )
                        ysb = xtiles.tile([P, D], F32, tag="ysb", bufs=2)
                        nc.vector.tensor_scalar_mul(ysb[:], psy[:], wc[:, t:t + 1])
                        nc.sync.dma_start(
                            out=y_out[t * P:(t + 1) * P, :], in_=ysb[:]
                        )
            tp_cm.__exit__(None, None, None)

    nc.compile()
    return nc


def make_in_maps(x, gate_w, w1, w2):
    """Build the 8 per-core input maps from the full tensors."""
    xf = np.ascontiguousarray(np.asarray(x, dtype=np.float32).reshape(N, D))
    gate_w = np.asarray(gate_w, dtype=np.float32)
    w1 = np.asarray(w1, dtype=np.float32)
    w2 = np.asarray(w2, dtype=np.float32)

    xT = np.ascontiguousarray(xf.T)

    in_maps = []
    for c in range(E):
        rot = [(c + j) % E for j in range(E)]
        gwT = gate_w[rot].T  # [D, E], column 0 = expert c
        gwt = np.ascontiguousarray(
            gwT.reshape(DC, P, E).transpose(1, 0, 2).reshape(P, DC * E)
        )
        # w1t[f, dp, dc*128 + fc] = w1[c][dc*128 + dp, f*128 + fc]
        w1t = np.ascontiguousarray(
            w1[c].reshape(DC, P, FT, P).transpose(2, 1, 0, 3).reshape(FT, P, DC * P)
        ).astype(ml_dtypes.bfloat16)
        in_maps.append({
            "xT": xT,
            "xrows": xf,
            "gwt": gwt,
            "w1t": w1t,
            "w2": np.ascontiguousarray(w2[c]).astype(ml_dtypes.bfloat16),
        })
    return in_maps


_cached = {}


def kernel(x, gate_w, w1, w2):
    key = "module"
    if key not in _cached:
        _cached[key] = build_module()
    nc = _cached[key]

    in_maps = make_in_maps(x, gate_w, w1, w2)
    res = run_bass_kernel_spmd(nc, in_maps, core_ids=list(range(E))).results

    out_flat = np.zeros((N, D), dtype=np.float32)
    for c in range(E):
        slots = res[c]["slot_out"]          # [P, NT], slot for token i*128+p
        slot_arr = np.ascontiguousarray(slots.T).reshape(N)
        sel = slot_arr < C
        y = res[c]["y_out"]                 # [C, D]
        out_flat[sel] += y[slot_arr[sel]]
    logits = res[0]["logits_out"]
    return out_flat.reshape(B, S, D), logits.reshape(B, S, E)
